# revision 36
# baseline (speedup 1.0000x reference)
"""Trainium2 Bass kernel for nn_Attention_52166672777669 (sparse_attention).

Math (reference):
    q  = LN(qx; g_q, b_q) @ wq.T                        # [256, 512]
    k  = LN(kx; g_k, b_k) @ wk.T                        # [256, 512, 512]
    S[q, kb, n] = (q[q] . k[kb, n]) / sqrt(512)         # masked, softmax over n
    out[q, kb, :] = sum_n P[q, kb, n] * kx[kb, n, :]    # [256, 256, 512]

Algebraic restructuring (exact up to fp rounding):
  S.T[n,q] = r_n * (kx[kb] @ Qg.T)[n,q]    per key-batch kb, with
  Qg = scale * g_k * (LNraw(qx) @ (wq_eff.T @ wk) + qb2), then row-centered:
  Qg -= mean_c(Qg)  — valid because sum_c (kx[n,c] - m_n) = 0: subtracting
  ubar*ones from a Qg row shifts S by exactly the LN mean-correction term.
  K projection GEMM never computed; LN(kx) never materialized.
  (q-only additive terms are dropped: softmax-invariant.)

Per key batch: PE 16 QK + 8 denom + 8 AV matmuls; ACT only Exp (single LUT
load for the whole kernel) + table-free Copy; DVE bn_stats + Newton rsqrt.
DMA: 2 packed loads (kx in two layouts, 4 KiB contiguous per partition) and
1 packed store per batch.

Sharding: Bk split across 8 cores (32 key-batches each). No collectives.
"""

import os
import sys

import numpy as np

for _p in ("/opt/trn_rl_repo",):
    if _p not in sys.path and os.path.isdir(_p):
        sys.path.insert(0, _p)

Bq, Bk, Nk, C = 256, 256, 512, 512
NCORES = 8
BKPC = Bk // NCORES  # key-batches per core
EPS = 1e-5
MASK_NEG = -100000.0
MAGIC = 0x5F3759DF

_cache = {}


def _build_nc():
    from contextlib import ExitStack

    import concourse.bacc as bacc
    import concourse.bass as bass
    import concourse.mybir as mybir
    import concourse.tile as tile

    f16 = mybir.dt.float16
    f32 = mybir.dt.float32
    u32 = mybir.dt.uint32
    i32 = mybir.dt.int32
    ts = bass.ts
    AF = mybir.ActivationFunctionType
    ALU = mybir.AluOpType

    nc = bacc.Bacc()
    QTc = Bq // 128

    qx_d = nc.declare_dram_parameter("qx_rows", [128, QTc * C], f16, isOutput=False)
    wqT_d = nc.declare_dram_parameter("wq_effT", [128, 4 * C], f16, isOutput=False)
    blob16_d = nc.declare_dram_parameter("blob16", [128, 256], f16, isOutput=False)
    blob32_d = nc.declare_dram_parameter("blob32", [128, 136], f32, isOutput=False)
    # packed layouts: [b][p][t][.] — 4 KiB contiguous per partition per batch
    kxn_d = nc.declare_dram_parameter("kxn", [BKPC, 128, 4 * C], f16, isOutput=False)
    kxt_d = nc.declare_dram_parameter("kxt", [BKPC, 128, 4 * Nk], f16, isOutput=False)
    # packed output: [b][p][mt][c] — host unpacks to [b, mt*128+p, c]
    out_d = nc.declare_dram_parameter("out", [BKPC, 128, 2 * C], f16, isOutput=True)

    NT = Nk // 128  # n tiles per key batch (4)
    CT = C // 128   # channel tiles (4)
    QT = Bq // 128  # query tiles (2)

    def newton_rsqrt(work, w, ncols, tagsuf):
        """DVE-only rsqrt of fp32 tile w [128, ncols]; returns fp32 tile."""
        y = work.tile([128, ncols], f32, tag=f"nwy{tagsuf}")
        t = work.tile([128, ncols], f32, tag=f"nwt{tagsuf}")
        yi = y[:].bitcast(u32)
        nc.vector.tensor_scalar(
            yi, w[:].bitcast(u32), 1, None, op0=ALU.logical_shift_right
        )
        nc.vector.tensor_scalar(
            y[:].bitcast(i32), y[:].bitcast(i32), -1, MAGIC, op0=ALU.mult, op1=ALU.add
        )
        for _ in range(2):
            nc.vector.tensor_mul(t[:], y[:], y[:])
            nc.vector.tensor_mul(t[:], t[:], w[:])
            nc.vector.tensor_scalar(t[:], t[:], -0.5, 1.5, op0=ALU.mult, op1=ALU.add)
            nc.vector.tensor_mul(y[:], y[:], t[:])
        return y

    def newton_rsqrt_strided(work, mvcat, ncols, tagsuf):
        """rsqrt of the var columns of a packed (mean,var) tile [128, 2*ncols]."""
        w = mvcat[:, 1 : 2 * ncols : 2]
        y = work.tile([128, ncols], f32, tag=f"nwy{tagsuf}")
        t = work.tile([128, ncols], f32, tag=f"nwt{tagsuf}")
        yi = y[:].bitcast(u32)
        nc.vector.tensor_scalar(yi, w.bitcast(u32), 1, None, op0=ALU.logical_shift_right)
        nc.vector.tensor_scalar(
            y[:].bitcast(i32), y[:].bitcast(i32), -1, MAGIC, op0=ALU.mult, op1=ALU.add
        )
        for _ in range(2):
            nc.vector.tensor_mul(t[:], y[:], y[:])
            nc.vector.tensor_mul(t[:], t[:], w)
            nc.vector.tensor_scalar(t[:], t[:], -0.5, 1.5, op0=ALU.mult, op1=ALU.add)
            nc.vector.tensor_mul(y[:], y[:], t[:])
        return y

    with tile.TileContext(nc) as tc, ExitStack() as ctx:
        consts = ctx.enter_context(tc.tile_pool(name="consts", bufs=1))
        work = ctx.enter_context(tc.tile_pool(name="work", bufs=2))
        ps = ctx.enter_context(tc.tile_pool(name="ps", bufs=1, space="PSUM"))

        # ------------- constants: 4 packed blob DMAs on gpsimd queues -------------
        blob16 = consts.tile([128, 256], f16)
        nc.gpsimd.dma_start(blob16[:], blob16_d[:, :])
        ident16 = blob16[:, 0:128]
        ones_col = blob16[:, 128:129]
        ones_row = blob16[0:1, 128:256]

        blob32 = consts.tile([128, 136], f32)
        nc.gpsimd.dma_start(blob32[:], blob32_d[:, :])
        colt = [blob32[:, ct * 34 : ct * 34 + 2] for ct in range(CT)]
        mbt = [blob32[:, ct * 34 + 2 : ct * 34 + 34] for ct in range(CT)]

        wqT_all = consts.tile([128, 4 * C], f16)
        nc.sync.dma_start(wqT_all[:], wqT_d[:, :])
        wqT = [wqT_all[:, ci * C : (ci + 1) * C] for ci in range(CT)]

        # single ACT LUT load for the whole kernel: one dummy Exp up front
        dummy = work.tile([128, 1], f16, tag="dummy")
        nc.scalar.activation(
            dummy[:], colt[0][:, 0:1], AF.Exp, bias=colt[0][:, 0:1], scale=0.0
        )

        # ---------------- setup: Qg.T (centered) ----------------
        qx_all = work.tile([128, QT * C], f16, tag="qx")
        nc.sync.dma_start(qx_all[:], qx_d[:, :])
        lnq = []
        for qt in range(QT):
            qx_t = qx_all[:, qt * C : (qt + 1) * C]
            st6 = work.tile([128, 6], f32, tag="qst6")
            nc.vector.bn_stats(st6[:], qx_t)
            mv = work.tile([128, 2], f32, tag=f"qmv{qt}")
            nc.vector.bn_aggr(mv[:], st6[:])
            wvar = work.tile([128, 1], f32, tag=f"qw{qt}")
            nc.vector.tensor_scalar(wvar[:], mv[:, 1:2], EPS, None, op0=ALU.add)
            r = newton_rsqrt(work, wvar, 1, f"q{qt}")
            ln = consts.tile([128, C], f16, tag=f"lnq{qt}")
            nc.vector.tensor_scalar(
                ln[:], qx_t, mv[:, 0:1], r[:], op0=ALU.subtract, op1=ALU.mult
            )
            lnq.append(ln)

        # transpose LN(qx) -> lnqT [c, q] tiles  (PE transpose + ACT copies)
        lnqT = []
        for ct in range(CT):
            t = consts.tile([128, Bq], f16, tag=f"lnqT{ct}")
            lnqT.append(t)
        for ct in range(CT):
            for qt in range(QT):
                pt = ps.tile([128, 128], f16, tag="psm", bufs=1)
                nc.tensor.transpose(pt[:], lnq[qt][:, ts(ct, 128)], ident16)
                nc.scalar.copy(lnqT[ct][:, ts(qt, 128)], pt[:])

        # QgT_raw[c', q] = ((wcomb.T @ lnqT) + qb2) * gk_scale  (per-part. c')
        qgT_raw = []
        for cp in range(CT):
            pq = ps.tile([128, Bq], f32, tag="psa", bufs=3)
            for ci in range(CT):
                nc.tensor.matmul(
                    pq[:],
                    wqT[ci][:, ts(cp, 128)],
                    lnqT[ci][:],
                    start=(ci == 0),
                    stop=(ci == CT - 1),
                )
            qg = work.tile([128, Bq], f16, tag=f"qgTr{cp}")
            nc.vector.tensor_scalar(
                qg[:],
                pq[:],
                colt[cp][:, 0:1],
                colt[cp][:, 1:2],
                op0=ALU.add,
                op1=ALU.mult,
            )
            qgT_raw.append(qg)

        # negubar[q] = -mean_c' QgT_raw[c', q]
        pu = ps.tile([1, Bq], f32, tag="psa", bufs=3)
        for cp in range(CT):
            nc.tensor.matmul(
                pu[:], ones_col, qgT_raw[cp][:], start=(cp == 0), stop=(cp == CT - 1)
            )
        negubar = consts.tile([1, Bq], f16)
        nc.scalar.mul(negubar[:], pu[:], -1.0 / C)

        # center: QgT[c', q] = QgT_raw[c', q] - ubar[q]
        qgT = []
        for cp in range(CT):
            pc = ps.tile([128, Bq], f32, tag="psa", bufs=3)
            nc.tensor.matmul(pc[:], ident16, qgT_raw[cp][:], start=True, stop=False)
            nc.tensor.matmul(pc[:], ones_row, negubar[:], start=False, stop=True)
            qg = consts.tile([128, Bq], f16, tag=f"qgT{cp}")
            nc.scalar.copy(qg[:], pc[:])
            qgT.append(qg)

        # ---------------- main loop over key batches (groups of 4) ----------------
        GRP = 4
        for g in range(BKPC // GRP):
            kxns = []
            kxts = []
            for bi in range(GRP):
                b = g * GRP + bi
                kxn = work.tile([128, 4 * C], f16, tag=f"kxn{bi}", bufs=2)
                nc.sync.dma_start(kxn[:], kxn_d[b, :, :])
                kxt = work.tile([128, 4 * Nk], f16, tag=f"kxt{bi}", bufs=2)
                nc.sync.dma_start(kxt[:], kxt_d[b, :, :])
                kxns.append(kxn)
                kxts.append(kxt)

            # row stats for the whole group -> one Newton rsqrt (DVE).
            # bn_aggr writes (mean,var) pairs into one packed tile; Newton
            # runs on the strided var view. eps dropped on k-path (var ~ 1).
            mvcat = work.tile([128, 2 * GRP * NT], f32, tag="mvcat")
            for bi in range(GRP):
                for t in range(NT):
                    st6 = work.tile([128, 6], f32, tag="kst6", bufs=3)
                    nc.vector.bn_stats(st6[:], kxns[bi][:, ts(t, C)])
                    j = 2 * (bi * NT + t)
                    nc.vector.bn_aggr(mvcat[:, j : j + 2], st6[:])
            rcat = newton_rsqrt_strided(work, mvcat, GRP * NT, "k")

            for bi in range(GRP):
                b = g * GRP + bi
                kxn = kxns[bi]
                kxt = kxts[bi]

                # scores S.T[n, q] per n-tile; exp -> pT fp16
                pT = []
                for t in range(NT):
                    pa = ps.tile([128, Bq], f32, tag="psa", bufs=3)
                    for ci in range(CT):
                        nc.tensor.matmul(
                            pa[:],
                            kxt[:, ci * Nk + t * 128 : ci * Nk + (t + 1) * 128],
                            qgT[ci][:],
                            start=(ci == 0),
                            stop=(ci == CT - 1),
                        )
                    pe = work.tile([128, Bq], f16, tag=f"pT{t}")
                    nc.scalar.activation(
                        pe[:],
                        pa[:],
                        AF.Exp,
                        bias=mbt[t][:, b : b + 1],
                        scale=rcat[:, bi * NT + t : bi * NT + t + 1],
                    )
                    pT.append(pe)

                # denom + AV interleaved: same lhsT per (mt, t) pair, so the
                # denom's weight load hides behind the 512-col AV stream
                pd = ps.tile([128, QT], f32, tag="psd", bufs=2)
                osb = work.tile([128, 2 * C], f16, tag="osb", bufs=3)
                rd = work.tile([128, QT], f32, tag="rd")
                for mt in range(QT):
                    po = ps.tile([128, C], f32, tag="pso", bufs=2)
                    for t in range(NT):
                        lhs = pT[t][:, ts(mt, 128)]
                        nc.tensor.matmul(
                            pd[:, mt : mt + 1],
                            lhs,
                            ones_col,
                            start=(t == 0),
                            stop=(t == NT - 1),
                        )
                        nc.tensor.matmul(
                            po[:],
                            lhs,
                            kxn[:, ts(t, C)],
                            start=(t == 0),
                            stop=(t == NT - 1),
                        )
                    nc.vector.reciprocal(rd[:, mt : mt + 1], pd[:, mt : mt + 1])
                    nc.scalar.mul(osb[:, ts(mt, C)], po[:], rd[:, mt : mt + 1])
                nc.sync.dma_start(out_d[b, :, :], osb[:])

    nc.compile()
    return nc


def _prep_host(qx, kx, key_padding_mask, ln_q_g, ln_q_b, ln_k_g, ln_k_b, wq, wk):
    f32 = np.float32
    QT = Bq // 128
    CT = C // 128
    # packed [p, qt*C + c]
    qx_rows = np.ascontiguousarray(
        np.asarray(qx, np.float16)
        .reshape(QT, 128, C)
        .transpose(1, 0, 2)
        .reshape(128, QT * C)
    )
    wq32 = np.asarray(wq, f32)
    wk32 = np.asarray(wk, f32)
    g_q = np.asarray(ln_q_g, f32)
    b_q = np.asarray(ln_q_b, f32)
    wq_eff = wq32 * g_q[None, :]          # [c', a]
    wcomb = (wq_eff.T @ wk32).astype(np.float16)  # [a, c]
    # packed [p, ci*C + c'] : tile ci holds rows a = ci*128+p
    wcomb_p = np.ascontiguousarray(
        wcomb.reshape(CT, 128, C).transpose(1, 0, 2).reshape(128, CT * C)
    )
    qb2 = ((wq32 @ b_q) @ wk32).astype(f32)  # [c]
    gks = (np.asarray(ln_k_g, f32) * (C ** -0.5)).astype(f32)

    blob16 = np.zeros((128, 256), np.float16)
    blob16[:, 0:128] = np.eye(128, dtype=np.float16)
    blob16[:, 128:256] = 1.0

    kx16 = np.asarray(kx, np.float16)
    mask = np.asarray(key_padding_mask)
    in_maps = []
    for i in range(NCORES):
        sl = slice(i * BKPC, (i + 1) * BKPC)
        kxs = kx16[sl]  # [BKPC, Nk, C]
        # packed: [b][p][t][c] / [b][p][ct][n]
        kxn = np.ascontiguousarray(
            kxs.reshape(BKPC, 4, 128, C).transpose(0, 2, 1, 3).reshape(BKPC, 128, 4 * C)
        )
        kxt = np.ascontiguousarray(
            kxs.transpose(0, 2, 1)
            .reshape(BKPC, 4, 128, Nk)
            .transpose(0, 2, 1, 3)
            .reshape(BKPC, 128, 4 * Nk)
        )
        # blob32: per ct: [qb2_col, gks_col, mbT(32 cols)] = 34 cols
        mbT = np.where(mask[sl], MASK_NEG, 0.0).astype(f32).T  # [Nk, BKPC]
        blob32 = np.zeros((128, 136), f32)
        for ct in range(4):
            rows = slice(ct * 128, (ct + 1) * 128)
            blob32[:, ct * 34] = qb2[rows]
            blob32[:, ct * 34 + 1] = gks[rows]
            blob32[:, ct * 34 + 2 : ct * 34 + 34] = mbT[rows]
        in_maps.append(
            dict(
                qx_rows=qx_rows,
                wq_effT=wcomb_p,
                blob16=blob16,
                blob32=np.ascontiguousarray(blob32),
                kxn=kxn,
                kxt=kxt,
            )
        )
    return in_maps


def _get_nc():
    if "nc" not in _cache:
        _cache["nc"] = _build_nc()
    return _cache["nc"]


def kernel(**inputs) -> np.ndarray:
    from concourse.bass_utils import run_bass_kernel_spmd

    nc = _get_nc()
    in_maps = _prep_host(**inputs)
    res = run_bass_kernel_spmd(nc, in_maps, list(range(NCORES)))
    outs = []
    for i in range(NCORES):
        o = res.results[i]["out"]  # [BKPC, 128, 2C] packed
        o = o.reshape(BKPC, 128, 2, C).transpose(0, 2, 1, 3).reshape(BKPC, Bq, C)
        outs.append(o.transpose(1, 0, 2))
    full = np.concatenate(outs, axis=1)
    return np.ascontiguousarray(full.astype(np.float16))


# revision 37
# speedup vs baseline: 1.0084x; 1.0084x over previous
"""Trainium2 Bass kernel for nn_Attention_52166672777669 (sparse_attention).

Math (reference):
    q  = LN(qx; g_q, b_q) @ wq.T                        # [256, 512]
    k  = LN(kx; g_k, b_k) @ wk.T                        # [256, 512, 512]
    S[q, kb, n] = (q[q] . k[kb, n]) / sqrt(512)         # masked, softmax over n
    out[q, kb, :] = sum_n P[q, kb, n] * kx[kb, n, :]    # [256, 256, 512]

Algebraic restructuring (exact up to fp rounding):
  S.T[n,q] = r_n * (kx[kb] @ Qg.T)[n,q]    per key-batch kb, with
  Qg = scale * g_k * (LNraw(qx) @ (wq_eff.T @ wk) + qb2), then row-centered:
  Qg -= mean_c(Qg)  — valid because sum_c (kx[n,c] - m_n) = 0: subtracting
  ubar*ones from a Qg row shifts S by exactly the LN mean-correction term.
  K projection GEMM never computed; LN(kx) never materialized.
  (q-only additive terms are dropped: softmax-invariant.)

Per key batch: PE 16 QK + 8 denom + 8 AV matmuls; ACT only Exp (single LUT
load for the whole kernel) + table-free Copy; DVE bn_stats + Newton rsqrt.
DMA: 2 packed loads (kx in two layouts, 4 KiB contiguous per partition) and
1 packed store per batch.

Sharding: Bk split across 8 cores (32 key-batches each). No collectives.
"""

import os
import sys

import numpy as np

for _p in ("/opt/trn_rl_repo",):
    if _p not in sys.path and os.path.isdir(_p):
        sys.path.insert(0, _p)

Bq, Bk, Nk, C = 256, 256, 512, 512
NCORES = 8
BKPC = Bk // NCORES  # key-batches per core
EPS = 1e-5
MASK_NEG = -100000.0
MAGIC = 0x5F3759DF

_cache = {}


def _build_nc():
    from contextlib import ExitStack

    import concourse.bacc as bacc
    import concourse.bass as bass
    import concourse.mybir as mybir
    import concourse.tile as tile

    f16 = mybir.dt.float16
    f32 = mybir.dt.float32
    u32 = mybir.dt.uint32
    i32 = mybir.dt.int32
    ts = bass.ts
    AF = mybir.ActivationFunctionType
    ALU = mybir.AluOpType

    nc = bacc.Bacc()
    QTc = Bq // 128

    qx_d = nc.declare_dram_parameter("qx_rows", [128, QTc * C], f16, isOutput=False)
    wqT_d = nc.declare_dram_parameter("wq_effT", [128, 4 * C], f16, isOutput=False)
    blob16_d = nc.declare_dram_parameter("blob16", [128, 256], f16, isOutput=False)
    blob32_d = nc.declare_dram_parameter("blob32", [128, 136], f32, isOutput=False)
    # packed layouts: [b][p][t][.] — 4 KiB contiguous per partition per batch
    kxn_d = nc.declare_dram_parameter("kxn", [BKPC, 128, 4 * C], f16, isOutput=False)
    kxt_d = nc.declare_dram_parameter("kxt", [BKPC, 128, 4 * Nk], f16, isOutput=False)
    # packed output: [b][p][mt][c] — host unpacks to [b, mt*128+p, c]
    out_d = nc.declare_dram_parameter("out", [BKPC, 128, 2 * C], f16, isOutput=True)

    NT = Nk // 128  # n tiles per key batch (4)
    CT = C // 128   # channel tiles (4)
    QT = Bq // 128  # query tiles (2)

    def newton_rsqrt(work, w, ncols, tagsuf):
        """DVE-only rsqrt of fp32 tile w [128, ncols]; returns fp32 tile."""
        y = work.tile([128, ncols], f32, tag=f"nwy{tagsuf}")
        t = work.tile([128, ncols], f32, tag=f"nwt{tagsuf}")
        yi = y[:].bitcast(u32)
        nc.vector.tensor_scalar(
            yi, w[:].bitcast(u32), 1, None, op0=ALU.logical_shift_right
        )
        nc.vector.tensor_scalar(
            y[:].bitcast(i32), y[:].bitcast(i32), -1, MAGIC, op0=ALU.mult, op1=ALU.add
        )
        for _ in range(2):
            nc.vector.tensor_mul(t[:], y[:], y[:])
            nc.vector.tensor_mul(t[:], t[:], w[:])
            nc.vector.tensor_scalar(t[:], t[:], -0.5, 1.5, op0=ALU.mult, op1=ALU.add)
            nc.vector.tensor_mul(y[:], y[:], t[:])
        return y

    def newton_rsqrt_strided(work, mvcat, ncols, tagsuf):
        """rsqrt of the var columns of a packed (mean,var) tile [128, 2*ncols]."""
        w = mvcat[:, 1 : 2 * ncols : 2]
        y = work.tile([128, ncols], f32, tag=f"nwy{tagsuf}")
        t = work.tile([128, ncols], f32, tag=f"nwt{tagsuf}")
        yi = y[:].bitcast(u32)
        nc.vector.tensor_scalar(yi, w.bitcast(u32), 1, None, op0=ALU.logical_shift_right)
        nc.vector.tensor_scalar(
            y[:].bitcast(i32), y[:].bitcast(i32), -1, MAGIC, op0=ALU.mult, op1=ALU.add
        )
        for _ in range(2):
            nc.vector.tensor_mul(t[:], y[:], y[:])
            nc.vector.tensor_mul(t[:], t[:], w)
            nc.vector.tensor_scalar(t[:], t[:], -0.5, 1.5, op0=ALU.mult, op1=ALU.add)
            nc.vector.tensor_mul(y[:], y[:], t[:])
        return y

    with tile.TileContext(nc) as tc, ExitStack() as ctx:
        consts = ctx.enter_context(tc.tile_pool(name="consts", bufs=1))
        work = ctx.enter_context(tc.tile_pool(name="work", bufs=2))
        ps = ctx.enter_context(tc.tile_pool(name="ps", bufs=1, space="PSUM"))

        # ------------- constants: 4 packed blob DMAs on gpsimd queues -------------
        blob16 = consts.tile([128, 256], f16)
        nc.gpsimd.dma_start(blob16[:], blob16_d[:, :])
        ident16 = blob16[:, 0:128]
        ones_col = blob16[:, 128:129]
        ones_row = blob16[0:1, 128:256]

        blob32 = consts.tile([128, 136], f32)
        nc.gpsimd.dma_start(blob32[:], blob32_d[:, :])
        colt = [blob32[:, ct * 34 : ct * 34 + 2] for ct in range(CT)]
        mbt = [blob32[:, ct * 34 + 2 : ct * 34 + 34] for ct in range(CT)]

        wqT_all = consts.tile([128, 4 * C], f16)
        nc.sync.dma_start(wqT_all[:], wqT_d[:, :])
        wqT = [wqT_all[:, ci * C : (ci + 1) * C] for ci in range(CT)]

        # single ACT LUT load for the whole kernel: one dummy Exp up front
        dummy = work.tile([128, 1], f16, tag="dummy")
        nc.scalar.activation(
            dummy[:], colt[0][:, 0:1], AF.Exp, bias=colt[0][:, 0:1], scale=0.0
        )

        # ---------------- setup: Qg.T (centered) ----------------
        qx_all = work.tile([128, QT * C], f16, tag="qx")
        nc.sync.dma_start(qx_all[:], qx_d[:, :])
        lnq = []
        for qt in range(QT):
            qx_t = qx_all[:, qt * C : (qt + 1) * C]
            st6 = work.tile([128, 6], f32, tag="qst6")
            nc.vector.bn_stats(st6[:], qx_t)
            mv = work.tile([128, 2], f32, tag=f"qmv{qt}")
            nc.vector.bn_aggr(mv[:], st6[:])
            wvar = work.tile([128, 1], f32, tag=f"qw{qt}")
            nc.vector.tensor_scalar(wvar[:], mv[:, 1:2], EPS, None, op0=ALU.add)
            r = newton_rsqrt(work, wvar, 1, f"q{qt}")
            ln = consts.tile([128, C], f16, tag=f"lnq{qt}")
            nc.vector.tensor_scalar(
                ln[:], qx_t, mv[:, 0:1], r[:], op0=ALU.subtract, op1=ALU.mult
            )
            lnq.append(ln)

        # transpose LN(qx) -> lnqT [c, q] tiles  (PE transpose + ACT copies)
        lnqT = []
        for ct in range(CT):
            t = consts.tile([128, Bq], f16, tag=f"lnqT{ct}")
            lnqT.append(t)
        for ct in range(CT):
            for qt in range(QT):
                pt = ps.tile([128, 128], f16, tag="psm", bufs=1)
                nc.tensor.transpose(pt[:], lnq[qt][:, ts(ct, 128)], ident16)
                nc.scalar.copy(lnqT[ct][:, ts(qt, 128)], pt[:])

        # QgT_raw[c', q] = ((wcomb.T @ lnqT) + qb2) * gk_scale  (per-part. c')
        qgT_raw = []
        for cp in range(CT):
            pq = ps.tile([128, Bq], f32, tag="psa", bufs=4)
            for ci in range(CT):
                nc.tensor.matmul(
                    pq[:],
                    wqT[ci][:, ts(cp, 128)],
                    lnqT[ci][:],
                    start=(ci == 0),
                    stop=(ci == CT - 1),
                )
            qg = work.tile([128, Bq], f16, tag=f"qgTr{cp}")
            nc.vector.tensor_scalar(
                qg[:],
                pq[:],
                colt[cp][:, 0:1],
                colt[cp][:, 1:2],
                op0=ALU.add,
                op1=ALU.mult,
            )
            qgT_raw.append(qg)

        # negubar[q] = -mean_c' QgT_raw[c', q]
        pu = ps.tile([1, Bq], f32, tag="psa", bufs=4)
        for cp in range(CT):
            nc.tensor.matmul(
                pu[:], ones_col, qgT_raw[cp][:], start=(cp == 0), stop=(cp == CT - 1)
            )
        negubar = consts.tile([1, Bq], f16)
        nc.scalar.mul(negubar[:], pu[:], -1.0 / C)

        # center: QgT[c', q] = QgT_raw[c', q] - ubar[q]
        qgT = []
        for cp in range(CT):
            pc = ps.tile([128, Bq], f32, tag="psa", bufs=4)
            nc.tensor.matmul(pc[:], ident16, qgT_raw[cp][:], start=True, stop=False)
            nc.tensor.matmul(pc[:], ones_row, negubar[:], start=False, stop=True)
            qg = consts.tile([128, Bq], f16, tag=f"qgT{cp}")
            nc.scalar.copy(qg[:], pc[:])
            qgT.append(qg)

        # ---------------- main loop over key batches (groups of 4) ----------------
        GRP = 4
        for g in range(BKPC // GRP):
            kxns = []
            kxts = []
            for bi in range(GRP):
                b = g * GRP + bi
                kxn = work.tile([128, 4 * C], f16, tag=f"kxn{bi}", bufs=3)
                nc.sync.dma_start(kxn[:], kxn_d[b, :, :])
                kxt = work.tile([128, 4 * Nk], f16, tag=f"kxt{bi}", bufs=3)
                nc.sync.dma_start(kxt[:], kxt_d[b, :, :])
                kxns.append(kxn)
                kxts.append(kxt)

            # row stats for the whole group -> one Newton rsqrt (DVE).
            # bn_aggr writes (mean,var) pairs into one packed tile; Newton
            # runs on the strided var view. eps dropped on k-path (var ~ 1).
            mvcat = work.tile([128, 2 * GRP * NT], f32, tag="mvcat")
            for bi in range(GRP):
                for t in range(NT):
                    st6 = work.tile([128, 6], f32, tag="kst6", bufs=3)
                    nc.vector.bn_stats(st6[:], kxns[bi][:, ts(t, C)])
                    j = 2 * (bi * NT + t)
                    nc.vector.bn_aggr(mvcat[:, j : j + 2], st6[:])
            rcat = newton_rsqrt_strided(work, mvcat, GRP * NT, "k")

            for bi in range(GRP):
                b = g * GRP + bi
                kxn = kxns[bi]
                kxt = kxts[bi]

                # scores S.T[n, q] per n-tile; exp -> pT fp16
                pT = []
                for t in range(NT):
                    pa = ps.tile([128, Bq], f32, tag="psa", bufs=4)
                    for ci in range(CT):
                        nc.tensor.matmul(
                            pa[:],
                            kxt[:, ci * Nk + t * 128 : ci * Nk + (t + 1) * 128],
                            qgT[ci][:],
                            start=(ci == 0),
                            stop=(ci == CT - 1),
                        )
                    pe = work.tile([128, Bq], f16, tag=f"pT{t}")
                    nc.scalar.activation(
                        pe[:],
                        pa[:],
                        AF.Exp,
                        bias=mbt[t][:, b : b + 1],
                        scale=rcat[:, bi * NT + t : bi * NT + t + 1],
                    )
                    pT.append(pe)

                # denom + AV interleaved: same lhsT per (mt, t) pair, so the
                # denom's weight load hides behind the 512-col AV stream
                pd = ps.tile([128, QT], f32, tag="psd", bufs=1)
                osb = work.tile([128, 2 * C], f16, tag="osb", bufs=3)
                rd = work.tile([128, QT], f32, tag="rd")
                for mt in range(QT):
                    po = ps.tile([128, C], f32, tag="pso", bufs=2)
                    for t in range(NT):
                        lhs = pT[t][:, ts(mt, 128)]
                        nc.tensor.matmul(
                            pd[:, mt : mt + 1],
                            lhs,
                            ones_col,
                            start=(t == 0),
                            stop=(t == NT - 1),
                        )
                        nc.tensor.matmul(
                            po[:],
                            lhs,
                            kxn[:, ts(t, C)],
                            start=(t == 0),
                            stop=(t == NT - 1),
                        )
                    nc.vector.reciprocal(rd[:, mt : mt + 1], pd[:, mt : mt + 1])
                    nc.scalar.mul(osb[:, ts(mt, C)], po[:], rd[:, mt : mt + 1])
                nc.sync.dma_start(out_d[b, :, :], osb[:])

    nc.compile()
    return nc


def _prep_host(qx, kx, key_padding_mask, ln_q_g, ln_q_b, ln_k_g, ln_k_b, wq, wk):
    f32 = np.float32
    QT = Bq // 128
    CT = C // 128
    # packed [p, qt*C + c]
    qx_rows = np.ascontiguousarray(
        np.asarray(qx, np.float16)
        .reshape(QT, 128, C)
        .transpose(1, 0, 2)
        .reshape(128, QT * C)
    )
    wq32 = np.asarray(wq, f32)
    wk32 = np.asarray(wk, f32)
    g_q = np.asarray(ln_q_g, f32)
    b_q = np.asarray(ln_q_b, f32)
    wq_eff = wq32 * g_q[None, :]          # [c', a]
    wcomb = (wq_eff.T @ wk32).astype(np.float16)  # [a, c]
    # packed [p, ci*C + c'] : tile ci holds rows a = ci*128+p
    wcomb_p = np.ascontiguousarray(
        wcomb.reshape(CT, 128, C).transpose(1, 0, 2).reshape(128, CT * C)
    )
    qb2 = ((wq32 @ b_q) @ wk32).astype(f32)  # [c]
    gks = (np.asarray(ln_k_g, f32) * (C ** -0.5)).astype(f32)

    blob16 = np.zeros((128, 256), np.float16)
    blob16[:, 0:128] = np.eye(128, dtype=np.float16)
    blob16[:, 128:256] = 1.0

    kx16 = np.asarray(kx, np.float16)
    mask = np.asarray(key_padding_mask)
    in_maps = []
    for i in range(NCORES):
        sl = slice(i * BKPC, (i + 1) * BKPC)
        kxs = kx16[sl]  # [BKPC, Nk, C]
        # packed: [b][p][t][c] / [b][p][ct][n]
        kxn = np.ascontiguousarray(
            kxs.reshape(BKPC, 4, 128, C).transpose(0, 2, 1, 3).reshape(BKPC, 128, 4 * C)
        )
        kxt = np.ascontiguousarray(
            kxs.transpose(0, 2, 1)
            .reshape(BKPC, 4, 128, Nk)
            .transpose(0, 2, 1, 3)
            .reshape(BKPC, 128, 4 * Nk)
        )
        # blob32: per ct: [qb2_col, gks_col, mbT(32 cols)] = 34 cols
        mbT = np.where(mask[sl], MASK_NEG, 0.0).astype(f32).T  # [Nk, BKPC]
        blob32 = np.zeros((128, 136), f32)
        for ct in range(4):
            rows = slice(ct * 128, (ct + 1) * 128)
            blob32[:, ct * 34] = qb2[rows]
            blob32[:, ct * 34 + 1] = gks[rows]
            blob32[:, ct * 34 + 2 : ct * 34 + 34] = mbT[rows]
        in_maps.append(
            dict(
                qx_rows=qx_rows,
                wq_effT=wcomb_p,
                blob16=blob16,
                blob32=np.ascontiguousarray(blob32),
                kxn=kxn,
                kxt=kxt,
            )
        )
    return in_maps


def _get_nc():
    if "nc" not in _cache:
        _cache["nc"] = _build_nc()
    return _cache["nc"]


def kernel(**inputs) -> np.ndarray:
    from concourse.bass_utils import run_bass_kernel_spmd

    nc = _get_nc()
    in_maps = _prep_host(**inputs)
    res = run_bass_kernel_spmd(nc, in_maps, list(range(NCORES)))
    outs = []
    for i in range(NCORES):
        o = res.results[i]["out"]  # [BKPC, 128, 2C] packed
        o = o.reshape(BKPC, 128, 2, C).transpose(0, 2, 1, 3).reshape(BKPC, Bq, C)
        outs.append(o.transpose(1, 0, 2))
    full = np.concatenate(outs, axis=1)
    return np.ascontiguousarray(full.astype(np.float16))
